# revision 7
# baseline (speedup 1.0000x reference)
"""Trainium2 Bass kernel for nn_ExhustiveContrastiveLoss.

Reference computation (N=8192, D=512, fp32):
    xd = normalize(embed_data); xl = normalize(embed_label)
    f2f = xd @ xd.T with diagonal removed; e2p = xd @ xl.T (full)
    per-strip row max subtracted before exp (the two strips use DIFFERENT
    maxes inside the same num/den sums, so the maxes are semantically
    load-bearing, not just numerics)
    num = sum(pos * e2p_logits) + sum(pos_nd * f2f_logits)
    den = sum(e2p_logits) + sum(f2f_logits)
    loss = -mean(log(num / den))

Sharding: 4x2 grid over 8 cores. Core k: row shard r = k % 4 (2048 rows of
xd), col shard c = k // 4 (4096 columns of the similarity matrices).
Each core computes, for its [2048, 4096] block of both strips and each row:
    C' = sum_j exp((S - 1)/T)        (shift-1 denominator partial)
    M  = max_j S                     (shard row max, via max_j l)
The NUMERATOR is computed on the host: positives are only the same-label
pairs (~N^2/1000 of all pairs), so A' = sum_{j in pos} exp((S - 1)/T) costs
O(N * D * avg_class_size) — same order as the input normalize — and is
exact (fp32 dots, fp64 exp-sums).  The host combines shards exactly as the
reference would:
    num = we*Ae' + wf*Af',  den = we*Ce' + wf*Cf',  wx = exp((1 - Mx)/T)
(the fixed shift of 1.0 >= max cos keeps exp in [3.8e-13, 1] — fp32 safe).

Matmuls run in fp8 (e4m3, inputs pre-scaled by 16 so entries clear the
subnormal cutoff; the activation rescales by 1/256 for free) with
MatmulPerfMode.DoubleRow: contraction 256 per instruction at 2 MAC/cell/cyc,
halving PE time vs fp32r. The f2f diagonal is removed by accumulating -1e9
into the diagonal PSUM cells via one extra bf16 identity-weight matmul
whose rhs is a per-core host input (all zeros on cores whose block does not
contain the diagonal), so the single SPMD program stays core-uniform.

DMA: inputs stream on TWO DGE queues (sync HWDGE for lhs + f2f rhs, gpsimd
SWDGE for masks + e2p rhs) with fused 3-dim APs (one dma_start per half
chunk instead of one per k-tile) so descriptor generation doesn't serialize
the pipeline head.

Device work: all O(N^2) matmul FLOPs at fp8 rate, exp + row-sum on ACT,
row-max tree on DVE. Host work is O(N*D) input prep + O(N*D*8) numerator +
O(N) final combine.
"""

import os

os.environ.setdefault("MYCRO_LOCAL_CACHE", "1")

import numpy as np

import concourse.bass as bass
import concourse.bacc as bacc
import concourse.tile as tile
from concourse import mybir
from concourse.bass_utils import run_bass_kernel_spmd

# Problem constants (hardcoded per harness contract).
N, D = 8192, 512
NCORES = 8
RGRID, CGRID = 4, 2          # 4 row shards x 2 col shards
R = N // RGRID               # 2048 rows per core
C = N // CGRID               # 4096 cols per core
NIT = R // 128               # 16 row tiles
CHUNK = 2048                 # col chunk processed per inner step
NCTP = C // CHUNK            # 2 col chunks
NKT = 4                      # contraction k-tiles (512 = 4 x 128)
TEMP = 0.07
EPS = 1e-8
SHIFT = 1.0                  # fixed exp shift; cos sim <= 1
FP8_SCALE = 16.0             # embeddings pre-scaled so fp8 entries are normal
NEG = -1.0e9

F32 = mybir.dt.float32
BF16 = mybir.dt.bfloat16
FP8 = mybir.dt.float8e4
AX = mybir.AxisListType
OP = mybir.AluOpType
AF = mybir.ActivationFunctionType
DR = mybir.MatmulPerfMode.DoubleRow


def build_nc():
    nc = bacc.Bacc(
        "TRN2",
        target_bir_lowering=False,
        debug=False,
        num_devices=NCORES,
    )

    lhs = nc.declare_dram_parameter("lhs", [D, R], FP8, isOutput=False)
    rhsD = nc.declare_dram_parameter("rhsD", [D, C], FP8, isOutput=False)
    rhsL = nc.declare_dram_parameter("rhsL", [D, C], FP8, isOutput=False)
    mA = nc.declare_dram_parameter("mA", [128, 128], BF16, isOutput=False)
    mB = nc.declare_dram_parameter("mB", [128, 128], BF16, isOutput=False)
    eyeK = nc.declare_dram_parameter("eyeK", [128, 128], BF16, isOutput=False)

    outs = {
        name: nc.declare_dram_parameter(name, [128, NIT], F32, isOutput=True)
        for name in ("df", "mf", "de", "me")
    }

    with tile.TileContext(nc) as tc:
        with (
            tc.tile_pool(name="const", bufs=1) as const,
            tc.tile_pool(name="rhsp", bufs=2) as rhsp,
            tc.tile_pool(name="psum", bufs=2, space="PSUM") as psum,
            tc.tile_pool(name="lp", bufs=4) as lp,
            tc.tile_pool(name="mtp", bufs=1) as mtp,
            tc.tile_pool(name="statp", bufs=1) as statp,
            tc.tile_pool(name="outp", bufs=1) as outp,
        ):
            dmaF = nc.sync       # HWDGE queue: lhs + f2f rhs
            dmaE = nc.gpsimd     # SWDGE queue: masks + e2p rhs

            def kt_view(dram, gs):
                # [D, cols] slice -> [128, NKT, cols] (k-tile as middle dim)
                return dram[:, gs].rearrange("(q p) c -> p q c", q=NKT)

            # lhs tile [128, 4, R]; first 512 cols first so the first
            # matmuls only wait for a small DMA
            lhs_sb = const.tile([128, NKT, R], FP8, tag="lhs")
            dmaF.dma_start(out=lhs_sb[:, :, 0:512], in_=kt_view(lhs, slice(0, 512)))

            # masks: [128, 896] zero tiles with the NEG-diag [128, 128]
            # block DMAed into cols 384:512 (tiny transfers, early arrival)
            eye_sb = const.tile([128, 128], BF16, tag="eyeK")
            dmaE.dma_start(out=eye_sb, in_=eyeK[:, :])
            mA_sb = const.tile([128, 896], BF16, tag="mA")
            mB_sb = const.tile([128, 896], BF16, tag="mB")
            nc.gpsimd.memset(mA_sb, 0.0)
            nc.gpsimd.memset(mB_sb, 0.0)
            dmaE.dma_start(out=mA_sb[:, 384:512], in_=mA[:, :])
            dmaE.dma_start(out=mB_sb[:, 384:512], in_=mB[:, :])
            bias_sb = const.tile([128, 1], F32, tag="expbias")
            nc.vector.memset(bias_sb, -SHIFT / TEMP)

            stats = {}
            for sname in ("f", "e"):
                for kind in ("d", "m"):
                    t = statp.tile(
                        [128, NIT * NCTP], F32,
                        tag=f"st_{kind}{sname}", name=f"st_{kind}{sname}",
                    )
                    stats[kind + sname] = t

            nnt = CHUNK // 512
            for ctp in range(NCTP):
                # stream this chunk of both rhs matrices, in halves so the
                # first matmuls only wait for ~half the bytes
                rhs_t = {}
                for mname in ("f", "e"):
                    rhs_t[mname] = rhsp.tile(
                        [128, NKT, CHUNK], FP8,
                        tag=f"rhs_{mname}", name=f"rhs_{mname}",
                    )
                for mname, dram, dq in (("f", rhsD, dmaF), ("e", rhsL, dmaE)):
                    if ctp == 0:
                        # halves: the first matmuls wait for fewer bytes
                        for half in range(2):
                            hs = slice(half * (CHUNK // 2),
                                       (half + 1) * (CHUNK // 2))
                            gs = slice(ctp * CHUNK + half * (CHUNK // 2),
                                       ctp * CHUNK + (half + 1) * (CHUNK // 2))
                            dq.dma_start(
                                out=rhs_t[mname][:, :, hs], in_=kt_view(dram, gs)
                            )
                    else:
                        # steady state: one transfer, 2 KiB contiguous lines
                        gs = slice(ctp * CHUNK, (ctp + 1) * CHUNK)
                        dq.dma_start(out=rhs_t[mname], in_=kt_view(dram, gs))
                if ctp == 0:
                    dmaF.dma_start(
                        out=lhs_sb[:, :, 512:R], in_=kt_view(lhs, slice(512, R))
                    )

                for it in range(NIT):
                    slot = slice(it * NCTP + ctp, it * NCTP + ctp + 1)

                    for sname in ("f", "e"):
                        ps = psum.tile([128, CHUNK], F32, tag="ps")
                        # kp-outer: nt-regions of one kp share the stationary
                        # weights, so LDWEIGHTS amortizes
                        for nt in range(nnt):
                            nc.tensor.matmul(
                                ps[:, nt * 512:(nt + 1) * 512],
                                lhsT=lhs_sb[:, 0:2, it * 128:(it + 1) * 128],
                                rhs=rhs_t[sname][:, 0:2, nt * 512:(nt + 1) * 512],
                                start=True,
                                stop=False,
                                perf_mode=DR,
                            )
                        for nt in range(nnt):
                            ct = ctp * nnt + nt
                            # diag mask: variant A at ct = it//4 (even row
                            # shard), variant B at ct = 4 + it//4 (odd)
                            mask_sb = None
                            if sname == "f":
                                if ct == it // 4:
                                    mask_sb = mA_sb
                                elif ct == 4 + it // 4:
                                    mask_sb = mB_sb
                            reg = ps[:, nt * 512:(nt + 1) * 512]
                            nc.tensor.matmul(
                                reg,
                                lhsT=lhs_sb[:, 2:4, it * 128:(it + 1) * 128],
                                rhs=rhs_t[sname][:, 2:4, nt * 512:(nt + 1) * 512],
                                start=False,
                                stop=(mask_sb is None),
                                perf_mode=DR,
                            )
                            if mask_sb is not None:
                                start_col = 384 - 128 * (it % 4)
                                nc.tensor.matmul(
                                    reg,
                                    lhsT=eye_sb,
                                    rhs=mask_sb[:, start_col:start_col + 512],
                                    start=False,
                                    stop=True,
                                )

                        l_t = lp.tile([128, CHUNK], BF16, tag="l")
                        nc.scalar.activation(
                            out=l_t,
                            in_=ps,
                            func=AF.Exp,
                            bias=bias_sb,
                            scale=1.0 / (TEMP * FP8_SCALE * FP8_SCALE),
                            accum_out=stats["d" + sname][:, slot],
                        )
                        # row max of l_t via 2x bf16 TT tree; the host
                        # recovers Ms = SHIFT + T*ln(max_l)
                        m1 = mtp.tile([128, CHUNK // 2], BF16, tag="m1")
                        nc.vector.tensor_tensor(
                            out=m1, in0=l_t[:, :CHUNK // 2],
                            in1=l_t[:, CHUNK // 2:], op=OP.max,
                        )
                        m2 = mtp.tile([128, CHUNK // 4], BF16, tag="m2")
                        nc.vector.tensor_tensor(
                            out=m2, in0=m1[:, :CHUNK // 4],
                            in1=m1[:, CHUNK // 4:], op=OP.max,
                        )
                        m3 = mtp.tile([128, CHUNK // 8], BF16, tag="m3")
                        nc.vector.tensor_tensor(
                            out=m3, in0=m2[:, :CHUNK // 8],
                            in1=m2[:, CHUNK // 8:], op=OP.max,
                        )
                        nc.vector.tensor_reduce(
                            out=stats["m" + sname][:, slot],
                            in_=m3,
                            axis=AX.X,
                            op=OP.max,
                        )

            # reduce per-ctp slots and ship out
            for sname in ("f", "e"):
                for kind, op in (("d", OP.add), ("m", OP.max)):
                    o = outp.tile(
                        [128, NIT], F32,
                        tag=f"o_{kind}{sname}", name=f"o_{kind}{sname}",
                    )
                    nc.vector.tensor_reduce(
                        out=o,
                        in_=stats[kind + sname].rearrange(
                            "p (a b) -> p a b", b=NCTP
                        ),
                        axis=AX.X,
                        op=op,
                    )
                    dmaF.dma_start(out=outs[kind + sname][:, :], in_=o)

    nc.finalize()
    return nc


_NC_CACHE = None


def _get_nc():
    global _NC_CACHE
    if _NC_CACHE is None:
        _NC_CACHE = build_nc()
    return _NC_CACHE


def _norm(x):
    n = np.sqrt(np.sum(x.astype(np.float64) ** 2, axis=1, keepdims=True))
    n = np.maximum(n, EPS)
    return (x / n).astype(np.float32)


def _host_numerators(xdn, xln, lab):
    """Exact per-row positive-pair sums A' = sum_pos exp((S-1)/T), fp64.

    f2f excludes the diagonal; e2p keeps it. Positives are same-label pairs,
    so this is ~N^2/1000 dot products — O(N*D*avg_class) host work.
    """
    n = xdn.shape[0]
    num_f = np.zeros(n, dtype=np.float64)
    num_e = np.zeros(n, dtype=np.float64)
    order = np.argsort(lab, kind="stable")
    sl = np.asarray(lab)[order]
    bounds = np.flatnonzero(np.diff(sl)) + 1
    starts = np.concatenate(([0], bounds, [n]))
    for a, b in zip(starts[:-1], starts[1:]):
        idx = order[a:b]
        Gd = xdn[idx]
        Gl = xln[idx]
        Sf = (Gd @ Gd.T).astype(np.float64)
        Se = (Gd @ Gl.T).astype(np.float64)
        Ef = np.exp((Sf - SHIFT) / TEMP)
        np.fill_diagonal(Ef, 0.0)
        Ee = np.exp((Se - SHIFT) / TEMP)
        num_f[idx] = Ef.sum(axis=1)
        num_e[idx] = Ee.sum(axis=1)
    return num_f, num_e


def _prep_inputs(embed_data, embed_label, label):
    import ml_dtypes

    xd = np.asarray(embed_data, dtype=np.float32)
    xl = np.asarray(embed_label, dtype=np.float32)
    lab = np.asarray(label)

    xdn = _norm(xd)
    xln = _norm(xl)

    num_f, num_e = _host_numerators(xdn, xln, lab)

    xdT8 = np.ascontiguousarray(
        (xdn.T * FP8_SCALE).astype(ml_dtypes.float8_e4m3)
    )  # [D, N]
    xlT8 = np.ascontiguousarray(
        (xln.T * FP8_SCALE).astype(ml_dtypes.float8_e4m3)
    )

    eyeK = np.eye(128, dtype=ml_dtypes.bfloat16)

    in_maps = []
    for k in range(NCORES):
        r, c = k % RGRID, k // RGRID
        rows = slice(R * r, R * (r + 1))
        cols = slice(C * c, C * (c + 1))

        mAb = np.zeros((128, 128), dtype=ml_dtypes.bfloat16)
        mBb = np.zeros((128, 128), dtype=ml_dtypes.bfloat16)
        if r // 2 == c:
            tgt = mAb if r % 2 == 0 else mBb
            tgt[np.arange(128), np.arange(128)] = NEG
        in_maps.append({
            "lhs": np.ascontiguousarray(xdT8[:, rows]),
            "rhsD": np.ascontiguousarray(xdT8[:, cols]),
            "rhsL": np.ascontiguousarray(xlT8[:, cols]),
            "mA": mAb,
            "mB": mBb,
            "eyeK": eyeK,
        })
    return in_maps, (num_f, num_e)


def _combine(results, host_aux):
    """Host combine of per-core shard stats + host numerators (fp64)."""
    num_f_host, num_e_host = host_aux

    # stats[name][r][c] = [128, NIT]; row g = 2048 r + 128 it + p
    def get(name):
        out = np.empty((RGRID, CGRID, 128, NIT), dtype=np.float64)
        for k in range(NCORES):
            r, c = k % RGRID, k // RGRID
            out[r, c] = results[k][name].astype(np.float64)
        return out

    df, mf = get("df"), get("mf")
    de, me = get("de"), get("me")

    # mf/me hold max_j l' = exp((Ms - SHIFT)/T); the reference weight
    # e^{(SHIFT - Ms)/T} is just its reciprocal.
    Mlf = np.max(mf, axis=1)           # [RGRID, 128, NIT]
    Mle = np.max(me, axis=1)
    Cf = np.sum(df, axis=1)
    Ce = np.sum(de, axis=1)

    # host numerators: A[g] with g = 2048 r + 128 it + p -> [r, p, it]
    Af = num_f_host.reshape(RGRID, NIT, 128).transpose(0, 2, 1)
    Ae = num_e_host.reshape(RGRID, NIT, 128).transpose(0, 2, 1)

    wf = 1.0 / Mlf
    we = 1.0 / Mle
    num = we * Ae + wf * Af
    den = we * Ce + wf * Cf
    row_loss = np.log(den) - np.log(num)
    return np.float32(np.mean(row_loss))


def kernel(embed_data, embed_label, label):
    nc = _get_nc()
    in_maps, host_aux = _prep_inputs(embed_data, embed_label, label)
    res = run_bass_kernel_spmd(nc, in_maps, list(range(NCORES)))
    return _combine(res.results, host_aux)


if __name__ == "__main__":
    rng = np.random.default_rng(0)
    ed = rng.standard_normal((N, D), dtype=np.float32)
    el = rng.standard_normal((N, D), dtype=np.float32)
    lb = rng.integers(0, 1000, N)
    print(kernel(ed, el, lb))


# revision 9
# speedup vs baseline: 1.0302x; 1.0302x over previous
"""Trainium2 Bass kernel for nn_ExhustiveContrastiveLoss.

Reference computation (N=8192, D=512, fp32):
    xd = normalize(embed_data); xl = normalize(embed_label)
    f2f = xd @ xd.T with diagonal removed; e2p = xd @ xl.T (full)
    per-strip row max subtracted before exp (the two strips use DIFFERENT
    maxes inside the same num/den sums, so the maxes are semantically
    load-bearing, not just numerics)
    num = sum(pos * e2p_logits) + sum(pos_nd * f2f_logits)
    den = sum(e2p_logits) + sum(f2f_logits)
    loss = -mean(log(num / den))

Sharding: 4x2 grid over 8 cores. Core k: row shard r = k % 4 (2048 rows of
xd), col shard c = k // 4 (4096 columns of the similarity matrices).
Each core computes, for its [2048, 4096] block of both strips and each row:
    C' = sum_j exp((S - 1)/T)        (shift-1 denominator partial)
    M  = max_j S                     (shard row max, via max_j l)
The NUMERATOR is computed on the host: positives are only the same-label
pairs (~N^2/1000 of all pairs), so A' = sum_{j in pos} exp((S - 1)/T) costs
O(N * D * avg_class_size) — same order as the input normalize — and is
exact (fp32 dots, fp64 exp-sums).  The host combines shards exactly as the
reference would:
    num = we*Ae' + wf*Af',  den = we*Ce' + wf*Cf',  wx = exp((1 - Mx)/T)
(the fixed shift of 1.0 >= max cos keeps exp in [3.8e-13, 1] — fp32 safe).

Matmuls run in fp8 (e4m3, inputs pre-scaled by 16 so entries clear the
subnormal cutoff; the activation rescales by 1/256 for free) with
MatmulPerfMode.DoubleRow: contraction 256 per instruction at 2 MAC/cell/cyc,
halving PE time vs fp32r. The f2f diagonal is removed by accumulating -1e9
into the diagonal PSUM cells via one extra bf16 identity-weight matmul
whose rhs is a per-core host input (all zeros on cores whose block does not
contain the diagonal), so the single SPMD program stays core-uniform.

DMA: inputs stream on TWO DGE queues (sync HWDGE for lhs + f2f rhs, gpsimd
SWDGE for masks + e2p rhs) with fused 3-dim APs (one dma_start per half
chunk instead of one per k-tile) so descriptor generation doesn't serialize
the pipeline head.

Device work: all O(N^2) matmul FLOPs at fp8 rate, exp + row-sum on ACT,
row-max tree on DVE. Host work is O(N*D) input prep + O(N*D*8) numerator +
O(N) final combine.
"""

import os

os.environ.setdefault("MYCRO_LOCAL_CACHE", "1")

import numpy as np

import concourse.bass as bass
import concourse.bacc as bacc
import concourse.tile as tile
from concourse import mybir
from concourse.bass_utils import run_bass_kernel_spmd

# Problem constants (hardcoded per harness contract).
N, D = 8192, 512
NCORES = 8
RGRID, CGRID = 4, 2          # 4 row shards x 2 col shards
R = N // RGRID               # 2048 rows per core
C = N // CGRID               # 4096 cols per core
NIT = R // 128               # 16 row tiles
CHUNK = 2048                 # col chunk processed per inner step
NCTP = C // CHUNK            # 2 col chunks
NKT = 4                      # contraction k-tiles (512 = 4 x 128)
TEMP = 0.07
EPS = 1e-8
SHIFT = 1.0                  # fixed exp shift; cos sim <= 1
FP8_SCALE = 16.0             # embeddings pre-scaled so fp8 entries are normal
NEG = -1.0e9

F32 = mybir.dt.float32
BF16 = mybir.dt.bfloat16
FP8 = mybir.dt.float8e4
AX = mybir.AxisListType
OP = mybir.AluOpType
AF = mybir.ActivationFunctionType
DR = mybir.MatmulPerfMode.DoubleRow


def build_nc():
    nc = bacc.Bacc(
        "TRN2",
        target_bir_lowering=False,
        debug=False,
        num_devices=NCORES,
    )

    lhs = nc.declare_dram_parameter("lhs", [D, R], FP8, isOutput=False)
    rhsD = nc.declare_dram_parameter("rhsD", [D, C], FP8, isOutput=False)
    rhsL = nc.declare_dram_parameter("rhsL", [D, C], FP8, isOutput=False)
    mA = nc.declare_dram_parameter("mA", [128, 128], BF16, isOutput=False)
    mB = nc.declare_dram_parameter("mB", [128, 128], BF16, isOutput=False)
    eyeK = nc.declare_dram_parameter("eyeK", [128, 128], BF16, isOutput=False)

    outs = {
        name: nc.declare_dram_parameter(name, [128, NIT], F32, isOutput=True)
        for name in ("df", "mf", "de", "me")
    }

    with tile.TileContext(nc) as tc:
        with (
            tc.tile_pool(name="const", bufs=1) as const,
            tc.tile_pool(name="rhsp", bufs=2) as rhsp,
            tc.tile_pool(name="psum", bufs=2, space="PSUM") as psum,
            tc.tile_pool(name="lp", bufs=4) as lp,
            tc.tile_pool(name="mtp", bufs=1) as mtp,
            tc.tile_pool(name="statp", bufs=1) as statp,
            tc.tile_pool(name="outp", bufs=1) as outp,
        ):
            dmaF = nc.sync       # HWDGE queue: lhs + f2f rhs + steady state
            dmaE = nc.scalar     # ACT HWDGE queue: masks + ctp0 e2p rhs
                                 # (ACT is idle during the pipeline head;
                                 # costs ~0.7us/dma of ACT queue time)

            def kt_view(dram, gs):
                # [D, cols] slice -> [128, NKT, cols] (k-tile as middle dim)
                return dram[:, gs].rearrange("(q p) c -> p q c", q=NKT)

            # lhs tile [128, 4, R]; first 512 cols first so the first
            # matmuls only wait for a small DMA
            lhs_sb = const.tile([128, NKT, R], FP8, tag="lhs")
            dmaF.dma_start(out=lhs_sb[:, :, 0:512], in_=kt_view(lhs, slice(0, 512)))

            # masks: [128, 896] zero tiles with the NEG-diag [128, 128]
            # block DMAed into cols 384:512 (tiny transfers, early arrival)
            eye_sb = const.tile([128, 128], BF16, tag="eyeK")
            dmaE.dma_start(out=eye_sb, in_=eyeK[:, :])
            mA_sb = const.tile([128, 896], BF16, tag="mA")
            mB_sb = const.tile([128, 896], BF16, tag="mB")
            nc.gpsimd.memset(mA_sb, 0.0)
            nc.gpsimd.memset(mB_sb, 0.0)
            dmaE.dma_start(out=mA_sb[:, 384:512], in_=mA[:, :])
            dmaE.dma_start(out=mB_sb[:, 384:512], in_=mB[:, :])
            bias_sb = const.tile([128, 1], F32, tag="expbias")
            nc.vector.memset(bias_sb, -SHIFT / TEMP)

            stats = {}
            for sname in ("f", "e"):
                for kind in ("d", "m"):
                    t = statp.tile(
                        [128, NIT * NCTP], F32,
                        tag=f"st_{kind}{sname}", name=f"st_{kind}{sname}",
                    )
                    stats[kind + sname] = t

            nnt = CHUNK // 512
            for ctp in range(NCTP):
                # stream this chunk of both rhs matrices, in halves so the
                # first matmuls only wait for ~half the bytes
                rhs_t = {}
                for mname in ("f", "e"):
                    rhs_t[mname] = rhsp.tile(
                        [128, NKT, CHUNK], FP8,
                        tag=f"rhs_{mname}", name=f"rhs_{mname}",
                    )
                gs = slice(ctp * CHUNK, (ctp + 1) * CHUNK)
                # ctp0: f and e stream on separate queues in parallel;
                # ctp1: both on the sync queue (ACT is busy by then)
                dmaF.dma_start(out=rhs_t["f"], in_=kt_view(rhsD, gs))
                (dmaE if ctp == 0 else dmaF).dma_start(
                    out=rhs_t["e"], in_=kt_view(rhsL, gs)
                )
                if ctp == 0:
                    dmaF.dma_start(
                        out=lhs_sb[:, :, 512:R], in_=kt_view(lhs, slice(512, R))
                    )

                for it in range(NIT):
                    slot = slice(it * NCTP + ctp, it * NCTP + ctp + 1)

                    for sname in ("f", "e"):
                        ps = psum.tile([128, CHUNK], F32, tag="ps")
                        # kp-outer: nt-regions of one kp share the stationary
                        # weights, so LDWEIGHTS amortizes
                        for nt in range(nnt):
                            nc.tensor.matmul(
                                ps[:, nt * 512:(nt + 1) * 512],
                                lhsT=lhs_sb[:, 0:2, it * 128:(it + 1) * 128],
                                rhs=rhs_t[sname][:, 0:2, nt * 512:(nt + 1) * 512],
                                start=True,
                                stop=False,
                                perf_mode=DR,
                            )
                        for nt in range(nnt):
                            ct = ctp * nnt + nt
                            # diag mask: variant A at ct = it//4 (even row
                            # shard), variant B at ct = 4 + it//4 (odd)
                            mask_sb = None
                            if sname == "f":
                                if ct == it // 4:
                                    mask_sb = mA_sb
                                elif ct == 4 + it // 4:
                                    mask_sb = mB_sb
                            reg = ps[:, nt * 512:(nt + 1) * 512]
                            nc.tensor.matmul(
                                reg,
                                lhsT=lhs_sb[:, 2:4, it * 128:(it + 1) * 128],
                                rhs=rhs_t[sname][:, 2:4, nt * 512:(nt + 1) * 512],
                                start=False,
                                stop=(mask_sb is None),
                                perf_mode=DR,
                            )
                            if mask_sb is not None:
                                start_col = 384 - 128 * (it % 4)
                                nc.tensor.matmul(
                                    reg,
                                    lhsT=eye_sb,
                                    rhs=mask_sb[:, start_col:start_col + 512],
                                    start=False,
                                    stop=True,
                                )

                        l_t = lp.tile([128, CHUNK], BF16, tag="l")
                        nc.scalar.activation(
                            out=l_t,
                            in_=ps,
                            func=AF.Exp,
                            bias=bias_sb,
                            scale=1.0 / (TEMP * FP8_SCALE * FP8_SCALE),
                            accum_out=stats["d" + sname][:, slot],
                        )
                        # row max of l_t via 2x bf16 TT tree; the host
                        # recovers Ms = SHIFT + T*ln(max_l)
                        m1 = mtp.tile([128, CHUNK // 2], BF16, tag="m1")
                        nc.vector.tensor_tensor(
                            out=m1, in0=l_t[:, :CHUNK // 2],
                            in1=l_t[:, CHUNK // 2:], op=OP.max,
                        )
                        m2 = mtp.tile([128, CHUNK // 4], BF16, tag="m2")
                        nc.vector.tensor_tensor(
                            out=m2, in0=m1[:, :CHUNK // 4],
                            in1=m1[:, CHUNK // 4:], op=OP.max,
                        )
                        m3 = mtp.tile([128, CHUNK // 8], BF16, tag="m3")
                        nc.vector.tensor_tensor(
                            out=m3, in0=m2[:, :CHUNK // 8],
                            in1=m2[:, CHUNK // 8:], op=OP.max,
                        )
                        nc.vector.tensor_reduce(
                            out=stats["m" + sname][:, slot],
                            in_=m3,
                            axis=AX.X,
                            op=OP.max,
                        )

            # reduce per-ctp slots and ship out
            for sname in ("f", "e"):
                for kind, op in (("d", OP.add), ("m", OP.max)):
                    o = outp.tile(
                        [128, NIT], F32,
                        tag=f"o_{kind}{sname}", name=f"o_{kind}{sname}",
                    )
                    nc.vector.tensor_reduce(
                        out=o,
                        in_=stats[kind + sname].rearrange(
                            "p (a b) -> p a b", b=NCTP
                        ),
                        axis=AX.X,
                        op=op,
                    )
                    dmaF.dma_start(out=outs[kind + sname][:, :], in_=o)

    nc.finalize()
    return nc


_NC_CACHE = None


def _get_nc():
    global _NC_CACHE
    if _NC_CACHE is None:
        _NC_CACHE = build_nc()
    return _NC_CACHE


def _norm(x):
    n = np.sqrt(np.sum(x.astype(np.float64) ** 2, axis=1, keepdims=True))
    n = np.maximum(n, EPS)
    return (x / n).astype(np.float32)


def _host_numerators(xdn, xln, lab):
    """Exact per-row positive-pair sums A' = sum_pos exp((S-1)/T), fp64.

    f2f excludes the diagonal; e2p keeps it. Positives are same-label pairs,
    so this is ~N^2/1000 dot products — O(N*D*avg_class) host work.
    """
    n = xdn.shape[0]
    num_f = np.zeros(n, dtype=np.float64)
    num_e = np.zeros(n, dtype=np.float64)
    order = np.argsort(lab, kind="stable")
    sl = np.asarray(lab)[order]
    bounds = np.flatnonzero(np.diff(sl)) + 1
    starts = np.concatenate(([0], bounds, [n]))
    for a, b in zip(starts[:-1], starts[1:]):
        idx = order[a:b]
        Gd = xdn[idx]
        Gl = xln[idx]
        Sf = (Gd @ Gd.T).astype(np.float64)
        Se = (Gd @ Gl.T).astype(np.float64)
        Ef = np.exp((Sf - SHIFT) / TEMP)
        np.fill_diagonal(Ef, 0.0)
        Ee = np.exp((Se - SHIFT) / TEMP)
        num_f[idx] = Ef.sum(axis=1)
        num_e[idx] = Ee.sum(axis=1)
    return num_f, num_e


def _prep_inputs(embed_data, embed_label, label):
    import ml_dtypes

    xd = np.asarray(embed_data, dtype=np.float32)
    xl = np.asarray(embed_label, dtype=np.float32)
    lab = np.asarray(label)

    xdn = _norm(xd)
    xln = _norm(xl)

    num_f, num_e = _host_numerators(xdn, xln, lab)

    xdT8 = np.ascontiguousarray(
        (xdn.T * FP8_SCALE).astype(ml_dtypes.float8_e4m3)
    )  # [D, N]
    xlT8 = np.ascontiguousarray(
        (xln.T * FP8_SCALE).astype(ml_dtypes.float8_e4m3)
    )

    eyeK = np.eye(128, dtype=ml_dtypes.bfloat16)

    in_maps = []
    for k in range(NCORES):
        r, c = k % RGRID, k // RGRID
        rows = slice(R * r, R * (r + 1))
        cols = slice(C * c, C * (c + 1))

        mAb = np.zeros((128, 128), dtype=ml_dtypes.bfloat16)
        mBb = np.zeros((128, 128), dtype=ml_dtypes.bfloat16)
        if r // 2 == c:
            tgt = mAb if r % 2 == 0 else mBb
            tgt[np.arange(128), np.arange(128)] = NEG
        in_maps.append({
            "lhs": np.ascontiguousarray(xdT8[:, rows]),
            "rhsD": np.ascontiguousarray(xdT8[:, cols]),
            "rhsL": np.ascontiguousarray(xlT8[:, cols]),
            "mA": mAb,
            "mB": mBb,
            "eyeK": eyeK,
        })
    return in_maps, (num_f, num_e)


def _combine(results, host_aux):
    """Host combine of per-core shard stats + host numerators (fp64)."""
    num_f_host, num_e_host = host_aux

    # stats[name][r][c] = [128, NIT]; row g = 2048 r + 128 it + p
    def get(name):
        out = np.empty((RGRID, CGRID, 128, NIT), dtype=np.float64)
        for k in range(NCORES):
            r, c = k % RGRID, k // RGRID
            out[r, c] = results[k][name].astype(np.float64)
        return out

    df, mf = get("df"), get("mf")
    de, me = get("de"), get("me")

    # mf/me hold max_j l' = exp((Ms - SHIFT)/T); the reference weight
    # e^{(SHIFT - Ms)/T} is just its reciprocal.
    Mlf = np.max(mf, axis=1)           # [RGRID, 128, NIT]
    Mle = np.max(me, axis=1)
    Cf = np.sum(df, axis=1)
    Ce = np.sum(de, axis=1)

    # host numerators: A[g] with g = 2048 r + 128 it + p -> [r, p, it]
    Af = num_f_host.reshape(RGRID, NIT, 128).transpose(0, 2, 1)
    Ae = num_e_host.reshape(RGRID, NIT, 128).transpose(0, 2, 1)

    wf = 1.0 / Mlf
    we = 1.0 / Mle
    num = we * Ae + wf * Af
    den = we * Ce + wf * Cf
    row_loss = np.log(den) - np.log(num)
    return np.float32(np.mean(row_loss))


def kernel(embed_data, embed_label, label):
    nc = _get_nc()
    in_maps, host_aux = _prep_inputs(embed_data, embed_label, label)
    res = run_bass_kernel_spmd(nc, in_maps, list(range(NCORES)))
    return _combine(res.results, host_aux)


if __name__ == "__main__":
    rng = np.random.default_rng(0)
    ed = rng.standard_normal((N, D), dtype=np.float32)
    el = rng.standard_normal((N, D), dtype=np.float32)
    lb = rng.integers(0, 1000, N)
    print(kernel(ed, el, lb))
